# revision 34
# baseline (speedup 1.0000x reference)
"""GPTQ 4-bit dequant + linear (x @ W.T + bias) on 8 Trainium2 NeuronCores.

Problem shapes (hardcoded):
  x       [4, 2048, 4096] f32   -> flattened to [8192, 4096], replicated
  qweight [16384, 512]    i32   (8x 4-bit nibbles per int32 along K)
  qzeros  [16384, 4]      i32
  scales  [16384, 32]     f32
  bias    [16384]         f32
  out     [4, 2048, 16384] f32

Sharding: column-parallel over out_features. Each of the 8 cores gets a
2048-row slab of qweight/qzeros/scales/bias, x replicated; outputs are
concatenated on the host along the feature axis.

Per-core kernel:
  Phase A: dequantize the int4 slab to bf16 W.T resident in SBUF
           ([128 kk, 32 c, 2048 n]). Nibble extract on DVE, per-group
           (q-z)*s split between DVE tensor_scalar and ACT
           activation(Identity, scale, bias), one batched xbar transpose
           per 128-row n-chunk.
  Phase B: per 128-token chunk: SWDGE cast-DMA x to bf16, ONE batched
           xbar transpose to [128k, 32c, 128t], then per 512-col n-block:
           32 PE matmuls accumulating a single PSUM bank, DVE PSUM+bias
           -> SBUF add, store on the ACT HWDGE ring.
  The first npro token chunks are processed n-block-major, interleaved
  with Phase A emission, so their matmuls execute inside Phase A's
  otherwise-idle PE windows (Tile tracks sub-tile deps on wT; each
  512-col n-block becomes available after 4 Phase-A chunks).
"""
import sys

for _p in ("/opt/trn_rl_repo", "/root/.axon_site/_ro/trn_rl_repo"):
    if _p not in sys.path:
        sys.path.append(_p)

import numpy as np
import concourse.bass as bass
import concourse.mybir as mybir
from concourse import tile, bacc
from concourse.bass_utils import run_bass_kernel_spmd

BF16 = mybir.dt.bfloat16
F32 = mybir.dt.float32
I32 = mybir.dt.int32

B, S, K, N = 4, 2048, 4096, 16384
T = B * S                      # 8192 tokens
NCORES = 8
NS = N // NCORES               # 2048 out features per core
PACK = 8
GS = 128                       # quant group size
G = K // GS                    # 32 groups == 32 k-chunks
TCH = 128                      # tokens per chunk
KC = K // 128                  # 32 k-chunks
MMN = 512                      # matmul moving free dim (one PSUM bank of f32)
NBLK = NS // MMN               # 4
NCH = NS // 128                # 16 weight n-chunks
HALF = K // 2                  # dequant processed in 2 half-chunks

_LSR = mybir.AluOpType.logical_shift_right
_AND = mybir.AluOpType.bitwise_and
_SUB = mybir.AluOpType.subtract
_MUL = mybir.AluOpType.mult
_ADD = mybir.AluOpType.add
IDENT = mybir.ActivationFunctionType.Identity

# fraction of the 32 per-group dequant ops on ACT (rest on DVE): 11 of every 16
ACT_MOD = 11


def build(t_total: int = T):
    nt = t_total // TCH
    nc = bacc.Bacc("TRN2", target_bir_lowering=False, debug=False)
    x_d = nc.dram_tensor("x", [t_total, K], F32, kind="ExternalInput")
    qw_d = nc.dram_tensor("qw", [NS, K // PACK], I32, kind="ExternalInput")
    qz_d = nc.dram_tensor("qz", [NS, G // PACK], I32, kind="ExternalInput")
    sc_d = nc.dram_tensor("sc", [NS, G], F32, kind="ExternalInput")
    b_d = nc.dram_tensor("b", [NS], F32, kind="ExternalInput")
    out_d = nc.dram_tensor("out", [t_total, NS], F32, kind="ExternalOutput")

    with tile.TileContext(nc) as tc:
        with (
            tc.tile_pool(name="wtp", bufs=1) as wtpool,
            tc.tile_pool(name="consts", bufs=1) as cpool,
            tc.tile_pool(name="aload", bufs=2) as apool,
            tc.tile_pool(name="anib", bufs=2) as nibpool,
            tc.tile_pool(name="awch", bufs=2) as wchpool,
            tc.tile_pool(name="bx", bufs=1) as bxpool,
            tc.tile_pool(name="bxt", bufs=3) as bxtpool,
            tc.tile_pool(name="bout", bufs=2) as bopool,
            tc.tile_pool(name="ps", bufs=8, space=bass.MemorySpace.PSUM) as pspool,
        ):
            # persistent dequantized W.T: [128 kk, 32 c, 2048 n] bf16
            wT = wtpool.tile([128, KC, NS], BF16)

            # helpers -------------------------------------------------
            def mm_block(ps_t, xT_t, nb):
                for c in range(KC):
                    nc.tensor.matmul(
                        ps_t[:], xT_t[:, c, :],
                        wT[:, c, nb * MMN:(nb + 1) * MMN],
                        start=(c == 0), stop=(c == KC - 1))

            def drain_store(ps_t, t0, nb):
                o_t = bopool.tile([128, MMN], F32, name="o_nb", tag="o_nb")
                nc.vector.tensor_tensor(
                    out=o_t[:], in0=ps_t[:],
                    in1=bias_t[:, nb * MMN:(nb + 1) * MMN], op=_ADD)
                nc.scalar.dma_start(
                    out_d[t0:t0 + TCH, nb * MMN:(nb + 1) * MMN], o_t[:])

            # prologue: stage the first npro token chunks' x loads early;
            # their matmuls run inside Phase A's idle PE windows
            npro = min(3, nt)
            pro_xb = []
            pro_xT = []
            for ti in range(npro):
                t0 = ti * TCH
                xb_t = bxpool.tile([128, K], BF16)
                nc.gpsimd.dma_start(xb_t[:], x_d[t0:t0 + TCH, :])  # f32->bf16
                pro_xb.append(xb_t)

            # bias broadcast to all 128 partitions: [128, 2048] bf16
            # (cast+broadcast during SWDGE DMA; added in f32 at PSUM drain)
            bias_t = cpool.tile([128, NS], BF16)
            b_row = b_d[:].rearrange("(o n) -> o n", o=1)
            b_bcast = bass.AP(tensor=b_row.tensor, offset=b_row.offset,
                              ap=[[0, 128], b_row.ap[1]])
            nc.gpsimd.dma_start(out=bias_t[:], in_=b_bcast)

            pend = []  # pending prologue drains: (ps_t, t0, nb)

            # ---- Phase A: dequantize weight slab, n-chunks of 128 rows
            for j in range(NCH):
                n0 = j * 128
                qw_t = apool.tile([128, K // PACK], I32)
                nc.sync.dma_start(qw_t[:], qw_d[n0:n0 + 128, :])
                qz_t = apool.tile([128, G // PACK], I32)
                nc.sync.dma_start(qz_t[:], qz_d[n0:n0 + 128, :])
                sc_t = apool.tile([128, G], F32)
                nc.sync.dma_start(sc_t[:], sc_d[n0:n0 + 128, :])
                if j < npro:
                    # interleave the prologue x transposes between weight loads
                    xT_t = bxtpool.tile([128, KC, TCH], BF16)
                    nc.sync.dma_start_transpose(xT_t[:], pro_xb[j][:])
                    pro_xT.append(xT_t)

                zi_t = apool.tile([128, G], I32)
                for i in range(PACK):
                    nc.vector.tensor_scalar(
                        out=zi_t[:, i::PACK], in0=qz_t[:],
                        scalar1=4 * i, scalar2=0xF, op0=_LSR, op1=_AND)
                z_t = apool.tile([128, G], F32)
                nc.vector.tensor_copy(z_t[:], zi_t[:])
                # zs = -z * s  (ACT bias operand)
                zs_t = apool.tile([128, G], F32)
                nc.vector.scalar_tensor_tensor(
                    out=zs_t[:], in0=z_t[:], scalar=-1.0, in1=sc_t[:],
                    op0=_MUL, op1=_MUL)

                w_t = wchpool.tile([128, K], BF16)
                for h in range(2):
                    w0 = h * (HALF // PACK)
                    nib_t = nibpool.tile([128, HALF], I32)
                    for i in range(PACK):
                        nc.vector.tensor_scalar(
                            out=nib_t[:, i::PACK],
                            in0=qw_t[:, w0:w0 + HALF // PACK],
                            scalar1=4 * i, scalar2=0xF, op0=_LSR, op1=_AND)
                    for gh in range(G // 2):
                        g = h * (G // 2) + gh
                        if (g % 16) < ACT_MOD:
                            # ACT: out = nib * s + (-z*s)
                            nc.scalar.activation(
                                w_t[:, g * GS:(g + 1) * GS],
                                nib_t[:, gh * GS:(gh + 1) * GS],
                                IDENT, bias=zs_t[:, g:g + 1],
                                scale=sc_t[:, g:g + 1])
                        else:
                            # DVE: out = (nib - z) * s
                            nc.vector.tensor_scalar(
                                out=w_t[:, g * GS:(g + 1) * GS],
                                in0=nib_t[:, gh * GS:(gh + 1) * GS],
                                scalar1=z_t[:, g:g + 1], scalar2=sc_t[:, g:g + 1],
                                op0=_SUB, op1=_MUL)

                    # batched xbar transpose per half, fired as soon as this
                    # half's dequant lands: w_t[:, h*2048:(h+1)*2048]
                    #   -> wT[:, h*16:(h+1)*16, n0:n0+128]
                    # (finer transpose granularity also tightens the single
                    # covering semaphore wait of the prologue matmul windows)
                    nc.sync.dma_start_transpose(
                        wT[:, h * (KC // 2):(h + 1) * (KC // 2), n0:n0 + 128],
                        w_t[:, h * HALF:(h + 1) * HALF])

                # prologue n-block scheduling: after window w = j//4 of 4
                # n-chunks is emitted, queue its matmuls; drain window w-1
                # (whose matmuls ran while window w was being produced)
                if j % 4 == 3:
                    w = j // 4
                    for d in pend:
                        drain_store(*d)
                    pend = []
                    for ti in range(npro):
                        ps_t = pspool.tile([128, MMN], F32,
                                           name="psnb", tag="psnb")
                        mm_block(ps_t, pro_xT[ti], w)
                        pend.append((ps_t, ti * TCH, w))

            for d in pend:
                drain_store(*d)
            pend = []

            # ---- Phase B: stream remaining tokens
            for ti in range(npro, nt):
                t0 = ti * TCH
                xb_t = bxpool.tile([128, K], BF16)
                nc.gpsimd.dma_start(xb_t[:], x_d[t0:t0 + TCH, :])  # cast
                xT_t = bxtpool.tile([128, KC, TCH], BF16)
                nc.sync.dma_start_transpose(xT_t[:], xb_t[:])
                for nb in range(NBLK):
                    ps_t = pspool.tile([128, MMN], F32, name="psnb", tag="psnb")
                    mm_block(ps_t, xT_t, nb)
                    drain_store(ps_t, t0, nb)

    nc.compile()
    return nc


_nc_cache = {}


def _get_nc(t_total: int = T):
    if t_total not in _nc_cache:
        _nc_cache[t_total] = build(t_total)
    return _nc_cache[t_total]


def kernel(x, qweight, qzeros, scales, bias, trace=False, t_total=T):
    xf = np.ascontiguousarray(
        x.reshape(-1, K)[:t_total].astype(np.float32, copy=False))
    in_maps = []
    for c in range(NCORES):
        sl = slice(c * NS, (c + 1) * NS)
        in_maps.append({
            "x": xf,
            "qw": np.ascontiguousarray(qweight[sl]),
            "qz": np.ascontiguousarray(qzeros[sl]),
            "sc": np.ascontiguousarray(scales[sl]),
            "b": np.ascontiguousarray(bias[sl]),
        })
    nc = _get_nc(t_total)
    res = run_bass_kernel_spmd(nc, in_maps, core_ids=list(range(NCORES)),
                               trace=trace)
    out = np.concatenate([r["out"] for r in res.results], axis=1)
    if t_total == T:
        out = out.reshape(B, S, N)
    out = out.astype(np.float32, copy=False)
    if trace:
        return out, res
    return out
